# revision 1
# baseline (speedup 1.0000x reference)
"""Trainium2 Bass kernel for a ViT-style EncoderBlock.

Problem: B=4, N=2048, D=768, H=12 heads (hd=64), FFN hidden 3072, fp32.
  y = x + proj(attn(LN1(x))) ;  out = y + fc2(gelu(fc1(LN2(y))))

Sharding (8 cores, zero communication): core c handles batch b=c//2 and
query-half s=c%2 (1024 query rows).  Each core receives the full batch-b
sequence (2048 rows) with its own query rows permuted to the front, computes
K/V over all 2048 rows, attention/FFN for its 1024 rows, and returns its
[1024, 768] slice of the output.  Host reassembles.

All matmuls run as float32r (full PE rate at fp32 storage).  Softmax skips
the max-subtraction (scores are ~N(0,1) after the 1/8 scale; exp is safe in
fp32) and folds the denominator into the AV matmul via a ones-column
appended to V (M=65 stationary).
"""

import sys

if "/opt/trn_rl_repo" not in sys.path:
    sys.path.insert(0, "/opt/trn_rl_repo")

import numpy as np

B, N, D = 4, 2048, 768
H, HD = 12, 64
HID = 4 * D
NQ = N // 2  # query rows per core
SCALE = HD ** -0.5
EPS = 1e-5

P = 128
DT = D // P          # 6 d-tiles
NQT = NQ // P        # 8 query tiles
NMT = N // P         # 16 kv tiles
HIDT = HID // P      # 24 hidden tiles
VW = HD + 1          # 65: V plus ones column

INPUT_NAMES = (
    "ln1_g", "ln1_b", "qkv_w", "proj_w", "proj_b",
    "ln2_g", "ln2_b", "fc1_w", "fc1_b", "fc2_w", "fc2_b",
)


def _layernorm_normalize(nc, small, work, x_sl, g_unused=None):
    """Row-wise LN of x_sl [128, 768] -> normalized tile [128, 768] (no g/b;
    those are applied post-transpose as per-partition scalars)."""
    import concourse.bass as bass
    from concourse import mybir

    st = small.tile([P, 2, 6], mybir.dt.float32, name="ln_st")
    for g in range(2):
        nc.vector.bn_stats(st[:, g, :], x_sl[:, g * 384:(g + 1) * 384])
    mv = small.tile([P, 2], mybir.dt.float32, name="ln_mv")
    nc.vector.bn_aggr(mv, st)
    ve = small.tile([P, 1], mybir.dt.float32, name="ln_ve")
    nc.vector.tensor_scalar_add(ve, mv[:, 1:2], EPS)
    sq = small.tile([P, 1], mybir.dt.float32, name="ln_sq")
    nc.scalar.sqrt(sq, ve)
    r0 = small.tile([P, 1], mybir.dt.float32, name="ln_r0")
    nc.vector.reciprocal(r0, sq)
    # one Newton step for rsqrt(ve): r = r0*(1.5 - 0.5*ve*r0^2)
    t = small.tile([P, 1], mybir.dt.float32, name="ln_t")
    nc.vector.tensor_scalar(t, r0, r0, ve, mybir.AluOpType.mult, mybir.AluOpType.mult)
    nc.vector.tensor_scalar(t, t, -0.5, 1.5, mybir.AluOpType.mult, mybir.AluOpType.add)
    r = small.tile([P, 1], mybir.dt.float32, name="ln_r")
    nc.vector.tensor_mul(r, r0, t)
    # nmr = -mu * r
    nmr = small.tile([P, 1], mybir.dt.float32, name="ln_nmr")
    nc.vector.tensor_scalar(
        nmr, mv[:, 0:1], r, -1.0, mybir.AluOpType.mult, mybir.AluOpType.mult
    )
    h = work.tile([P, D], mybir.dt.float32, name="ln_h")
    nc.vector.tensor_scalar(h, x_sl, r, nmr, mybir.AluOpType.mult, mybir.AluOpType.add)
    return h


def _encoder_body(tc, out_ap, aps):
    import concourse.bass as bass
    from concourse import mybir
    from concourse.masks import make_identity

    nc = tc.nc
    f32 = mybir.dt.float32
    f32r = mybir.dt.float32r
    bf16 = mybir.dt.bfloat16
    AF = mybir.ActivationFunctionType
    OP = mybir.AluOpType

    def mm(psum, lhsT, rhs, start, stop):
        nc.tensor.matmul(psum, lhsT, rhs, start=start, stop=stop)

    x = aps["x"]

    # ---------------- constants ----------------
    consts = tc.alloc_tile_pool(name="consts", bufs=1)
    ident = consts.tile([P, P], f32, name="ident")
    make_identity(nc, ident)
    g1c = consts.tile([P, DT], f32, name="g1c")
    b1c = consts.tile([P, DT], f32, name="b1c")
    g2c = consts.tile([P, DT], f32, name="g2c")
    b2c = consts.tile([P, DT], f32, name="b2c")
    f1bc = consts.tile([P, HIDT], f32, name="f1bc")
    nc.gpsimd.dma_start(out=g1c, in_=aps["ln1_g"].rearrange("(t p) -> p t", p=P))
    nc.gpsimd.dma_start(out=b1c, in_=aps["ln1_b"].rearrange("(t p) -> p t", p=P))
    nc.gpsimd.dma_start(out=g2c, in_=aps["ln2_g"].rearrange("(t p) -> p t", p=P))
    nc.gpsimd.dma_start(out=b2c, in_=aps["ln2_b"].rearrange("(t p) -> p t", p=P))
    nc.gpsimd.dma_start(out=f1bc, in_=aps["fc1_b"].rearrange("(t p) -> p t", p=P))
    pjb = consts.tile([P, D], f32, name="pjb")
    f2b = consts.tile([P, D], f32, name="f2b")
    nc.gpsimd.dma_start(out=pjb, in_=aps["proj_b"].partition_broadcast(P))
    nc.gpsimd.dma_start(out=f2b, in_=aps["fc2_b"].partition_broadcast(P))
    # f32r tiles cannot be memset directly (walrus ISA check) — memset an
    # f32 staging tile and copy (DVE copy casts/rounds to f32r).
    ones_f = consts.tile([P, HD], f32, name="ones_f")
    nc.vector.memset(ones_f, 1.0)
    ones64 = consts.tile([1, HD], f32r, name="ones64")
    nc.vector.tensor_copy(ones64, ones_f[0:1, :])
    identb = consts.tile([P, P], bf16, name="identb")
    nc.vector.tensor_copy(identb, ident)

    # Dedicated PE keep-warm: dep-free bf16 matmuls into a reserved PSUM bank.
    # The HAM clock gate re-throttles PE to 1.2 GHz across phase-boundary
    # bubbles; these matmuls hold the activity window open.
    warmp = tc.alloc_tile_pool(name="warmp", bufs=1, space="PSUM")
    warm_ps = warmp.tile([P, 512], f32, name="warm_ps")

    def warm(n):
        for _ in range(n):
            nc.tensor.matmul(warm_ps[:, 0:P], identb, identb, start=True, stop=True)

    def warm_dense(rhs):
        nc.tensor.matmul(warm_ps, identb, rhs, start=True, stop=True)

    # FFN weights: allocate early so the cast-DMAs prefetch during the
    # attention phase (DMA engines are idle there).
    f1wp = tc.alloc_tile_pool(name="f1w", bufs=1)
    w1a = f1wp.tile([P, DT, HID], bf16, name="w1a")
    nc.gpsimd.dma_start(out=w1a, in_=aps["fc1_w"].rearrange("(t p) c -> p t c", p=P))
    f2wp = tc.alloc_tile_pool(name="f2w", bufs=1)
    w2a = f2wp.tile([P, HIDT, D], bf16, name="w2a")
    nc.gpsimd.dma_start(out=w2a, in_=aps["fc2_w"].rearrange("(j p) d -> p j d", p=P))

    # ---------------- phase 1+2 : LN1 -> hT ; QKV ----------------
    qkv = tc.alloc_tile_pool(name="qkv", bufs=1)
    qT = qkv.tile([P, DT, NQ], bf16, name="qT")       # [qcol, nq] 6x[128,1024]
    kT = qkv.tile([P, DT, N], bf16, name="kT")        # [kcol, m]  6x[128,2048]
    V4 = qkv.tile([P, NMT, H, VW], bf16, name="V4")   # [m, mt, h, 65]

    hTp = tc.alloc_tile_pool(name="hTp", bufs=1)
    hT = hTp.tile([P, DT, N], bf16, name="hT")        # LN1(x)^T

    with tc.tile_pool(name="p1work", bufs=6) as work, \
         tc.tile_pool(name="p1small", bufs=8) as small, \
         tc.tile_pool(name="p1psum", bufs=4, space="PSUM") as psT:
        for i in range(NMT):
            xt = work.tile([P, D], f32, name="xt")
            nc.sync.dma_start(out=xt, in_=x[i * P:(i + 1) * P, :])
            h = _layernorm_normalize(nc, small, work, xt)
            for t in range(DT):
                ps = psT.tile([P, P], f32, name="trps")
                nc.tensor.transpose(ps, h[:, t * P:(t + 1) * P], ident)
                nc.scalar.activation(
                    hT[:, t, i * P:(i + 1) * P], ps, AF.Identity,
                    bias=b1c[:, t:t + 1], scale=g1c[:, t:t + 1],
                )

    # QKV projections (weights resident one of q/k/v at a time)
    with tc.tile_pool(name="wqkv", bufs=1) as wpool, \
         tc.tile_pool(name="p2psum", bufs=2, space="PSUM") as psQ:
        # --- Q: qT[c, nq] = Wq[d, c].T @ hT[d, nq] ---
        wq = wpool.tile([P, DT, D], bf16, name="wq", tag="w")
        nc.gpsimd.dma_start(
            out=wq, in_=aps["qkv_w"][:, 0:D].rearrange("(t p) c -> p t c", p=P)
        )
        for c in range(DT):
            for ch in range(2):
                ps = psQ.tile([P, 512], f32, name="qps", tag="ps")
                for t in range(DT):
                    mm(ps, wq[:, t, c * P:(c + 1) * P],
                       hT[:, t, ch * 512:(ch + 1) * 512], t == 0, t == DT - 1)
                nc.scalar.copy(qT[:, c, ch * 512:(ch + 1) * 512], ps)
        # --- K ---
        wk = wpool.tile([P, DT, D], bf16, name="wk", tag="w")
        nc.gpsimd.dma_start(
            out=wk, in_=aps["qkv_w"][:, D:2 * D].rearrange("(t p) c -> p t c", p=P)
        )
        for c in range(DT):
            for ch in range(4):
                ps = psQ.tile([P, 512], f32, name="kps", tag="ps")
                for t in range(DT):
                    mm(ps, wk[:, t, c * P:(c + 1) * P],
                       hT[:, t, ch * 512:(ch + 1) * 512], t == 0, t == DT - 1)
                nc.scalar.copy(kT[:, c, ch * 512:(ch + 1) * 512], ps)
        # --- V (natural layout [m, vcol]) + ones column ---
        wv = wpool.tile([P, DT, D], bf16, name="wv", tag="w")
        nc.gpsimd.dma_start(
            out=wv, in_=aps["qkv_w"][:, 2 * D:3 * D].rearrange("(t p) c -> p t c", p=P)
        )
        for mt in range(NMT):
            psv = psQ.tile([P, D], f32, name="vps", tag="psv")
            for t in range(DT):
                mm(psv[:, 0:512], hT[:, t, mt * P:(mt + 1) * P],
                   wv[:, t, 0:512], t == 0, t == DT - 1)
            for t in range(DT):
                mm(psv[:, 512:768], hT[:, t, mt * P:(mt + 1) * P],
                   wv[:, t, 512:768], t == 0, t == DT - 1)
            nc.scalar.copy(
                V4[:, mt, 0:8, 0:HD], psv[:, 0:512].rearrange("p (a b) -> p a b", a=8)
            )
            nc.scalar.copy(
                V4[:, mt, 8:12, 0:HD],
                psv[:, 512:768].rearrange("p (a b) -> p a b", a=4),
            )
            nc.vector.tensor_copy(
                V4[:, mt, :, HD:VW], ones_f[:, 0:H].rearrange("p (a b) -> p a b", b=1)
            )

    hTp.release()

    # ---------------- phase 3 : attention ----------------
    otp = tc.alloc_tile_pool(name="otp", bufs=1, side="right")
    OT = otp.tile([HD, H, NQ], bf16, name="OT")  # normalized O^T per head

    with tc.tile_pool(name="a_es", bufs=6) as esp, \
         tc.tile_pool(name="a_small", bufs=4) as asmall, \
         tc.tile_pool(name="a_psS", bufs=2, space="PSUM") as psS, \
         tc.tile_pool(name="a_psB", bufs=1, space="PSUM") as psB, \
         tc.tile_pool(name="a_psO", bufs=2, space="PSUM") as psO:
        warm(48)
        for j in range(H // 2):
            hA, hB = 2 * j, 2 * j + 1
            kTa, kTb = kT[0:HD, j, :], kT[HD:P, j, :]
            qTa, qTb = qT[0:HD, j, :], qT[HD:P, j, :]
            for ch in range(2):
                warm(2)
                cs = slice(ch * 512, (ch + 1) * 512)
                poA = psO.tile([VW, 512], f32, name="poA", tag="po")
                poB = psO.tile([VW, 512], f32, name="poB", tag="po")
                for mt in range(NMT):
                    msl = slice(mt * P, (mt + 1) * P)
                    ps = psS.tile([P, 1024], f32, name="sps")
                    # two heads on the two 64-row halves of the PE array
                    mm(ps[:, 0:512], kTa[:, msl], qTa[:, cs], True, True)
                    mm(ps[:, 512:1024], kTb[:, msl], qTb[:, cs], True, True)
                    es = esp.tile([P, 1024], bf16, name="es")
                    nc.scalar.activation(es, ps, AF.Exp, scale=SCALE)
                    mm(poA, V4[:, mt, hA, :], es[:, 0:512], mt == 0, mt == NMT - 1)
                    mm(poB, V4[:, mt, hB, :], es[:, 512:1024], mt == 0, mt == NMT - 1)
                    if mt % 2 == 0:
                        warm_dense(kT[:, 0, 0:512])
                posbA = asmall.tile([VW, 512], f32, name="posbA", tag="posbA", bufs=2)
                nc.vector.tensor_copy(posbA, poA)
                posbB = asmall.tile([VW, 512], f32, name="posbB", tag="posbB", bufs=2)
                nc.vector.tensor_copy(posbB, poB)
                for h, posb in ((hA, posbA), (hB, posbB)):
                    rd = asmall.tile([1, 512], f32r, name="rd", bufs=2)
                    with nc.allow_low_precision(reason="softmax denom recip"):
                        nc.vector.reciprocal(rd, posb[HD:VW, :])
                    rb = psB.tile([HD, 512], f32, name="rb")
                    mm(rb, ones64, rd, True, True)
                    ots = OT[:, h, cs]
                    nc.vector.tensor_tensor(ots, posb[0:HD, :], rb, OP.mult)

    qkv.release()

    # ---------------- phase 4 : proj + residual + LN2 -> x2T ----------------
    res1p = tc.alloc_tile_pool(name="res1p", bufs=1)
    res1 = res1p.tile([P, NQT, D], f32, name="res1")
    x2Tp = tc.alloc_tile_pool(name="x2Tp", bufs=1)
    x2T = x2Tp.tile([P, DT, NQ], bf16, name="x2T")

    with tc.tile_pool(name="pjw", bufs=1) as pjwp, \
         tc.tile_pool(name="p4work", bufs=6) as work4, \
         tc.tile_pool(name="p4small", bufs=8) as small4, \
         tc.tile_pool(name="p4psum", bufs=2, space="PSUM") as psP, \
         tc.tile_pool(name="p4psT", bufs=3, space="PSUM") as psT4:
        pjw = pjwp.tile([HD, H, D], bf16, name="pjw")
        nc.gpsimd.dma_start(out=pjw, in_=aps["proj_w"].rearrange("(h p) d -> p h d", p=HD))
        warm(16)
        for i in range(NQT):
            warm(6)
            psp = psP.tile([P, D], f32, name="psp")
            for h in range(H):
                mm(psp[:, 0:512], OT[:, h, i * P:(i + 1) * P], pjw[:, h, 0:512],
                   h == 0, h == H - 1)
            for h in range(H):
                mm(psp[:, 512:768], OT[:, h, i * P:(i + 1) * P], pjw[:, h, 512:768],
                   h == 0, h == H - 1)
            xr = work4.tile([P, D], f32, name="xr")
            nc.sync.dma_start(out=xr, in_=x[i * P:(i + 1) * P, :])
            r1 = res1[:, i, :]
            nc.vector.tensor_add(r1, psp, xr)
            nc.vector.tensor_add(r1, r1, pjb)
            h2 = _layernorm_normalize(nc, small4, work4, r1)
            for t in range(DT):
                ps = psT4.tile([P, P], f32, name="trps4")
                nc.tensor.transpose(ps, h2[:, t * P:(t + 1) * P], ident)
                nc.scalar.activation(
                    x2T[:, t, i * P:(i + 1) * P], ps, AF.Identity,
                    bias=b2c[:, t:t + 1], scale=g2c[:, t:t + 1],
                )
        otp.release()

        # pre-add fc2 bias into the residual accumulator
        for i in range(NQT):
            nc.vector.tensor_add(res1[:, i, :], res1[:, i, :], f2b)

    # ---------------- phase 5 : FFN fused per nq-chunk ----------------
    h1p = tc.alloc_tile_pool(name="h1p", bufs=1, side="right")
    warm(48)
    with tc.tile_pool(name="p5psum", bufs=2, space="PSUM") as psF, \
         tc.tile_pool(name="p6psum", bufs=2, space="PSUM") as ps2:
      for ch in range(2):
        h1 = h1p.tile([P, HIDT, 512], bf16, name="h1", tag="h1")
        for hc in range(HIDT):
            psf = psF.tile([P, 512], f32, name="psf")
            for t in range(DT):
                mm(psf, w1a[:, t, hc * P:(hc + 1) * P],
                   x2T[:, t, ch * 512:(ch + 1) * 512], t == 0, t == DT - 1)
            nc.scalar.activation(
                h1[:, hc, :], psf, AF.Gelu, bias=f1bc[:, hc:hc + 1]
            )
            if hc % 4 == 0:
                warm_dense(x2T[:, 0, 0:512])
        for i2 in range(4):
            i = ch * 4 + i2
            psq = ps2.tile([P, D], f32, name="psq")
            for j in range(HIDT):
                mm(psq[:, 0:512], h1[:, j, i2 * P:(i2 + 1) * P],
                   w2a[:, j, 0:512], j == 0, j == HIDT - 1)
            for j in range(HIDT):
                mm(psq[:, 512:768], h1[:, j, i2 * P:(i2 + 1) * P],
                   w2a[:, j, 512:768], j == 0, j == HIDT - 1)
            nc.vector.tensor_add(res1[:, i, :], res1[:, i, :], psq)
            warm_dense(h1[:, 0, :])
    x2Tp.release()
    h1p.release()

    # ---------------- output ----------------
    for i in range(NQT):
        nc.sync.dma_start(out=out_ap[i * P:(i + 1) * P, :], in_=res1[:, i, :])
    res1p.release()
    warmp.release()
    f2wp.release()
    f1wp.release()
    consts.release()


def build_nc(hoist_waits=True):
    import concourse.bass as bass
    import concourse.tile as tile
    from concourse import mybir

    f32 = mybir.dt.float32
    nc = bass.Bass("TRN2", target_bir_lowering=False, debug=False)
    aps = {"x": nc.dram_tensor("x", [N, D], f32, kind="ExternalInput").ap()}
    shapes = {
        "ln1_g": [D], "ln1_b": [D], "qkv_w": [D, 3 * D],
        "proj_w": [D, D], "proj_b": [D], "ln2_g": [D], "ln2_b": [D],
        "fc1_w": [D, HID], "fc1_b": [HID], "fc2_w": [HID, D], "fc2_b": [D],
    }
    for name in INPUT_NAMES:
        aps[name] = nc.dram_tensor(name, shapes[name], f32, kind="ExternalInput").ap()
    out_ap = nc.dram_tensor("out", [NQ, D], f32, kind="ExternalOutput").ap()
    with tile.TileContext(nc) as tc:
        _encoder_body(tc, out_ap, aps)
    if hoist_waits:
        _hoist_matmul_waits(nc)
    return nc


def _hoist_matmul_waits(nc):
    """walrus's LW-path matmuls (transpose / fp32 / f32r self-loading) accept
    only one embedded sync-wait.  Tile can attach two (one per producer
    engine).  Hoist all-but-one onto a standalone InstEventSemaphore placed
    just before the matmul in the same engine stream."""
    from concourse import mybir

    skip = (
        mybir.InstEventSemaphore,
        mybir.InstUnconditionalBranch,
    )
    for f in nc.m.functions:
        for bb in f.blocks:
            out = []
            for ins in bb.instructions:
                si = getattr(ins, "sync_info", None)
                if (
                    si is not None
                    and si.on_wait
                    and len(si.on_wait) > 1
                    and not isinstance(ins, skip)
                ):
                    for k, wait in enumerate(si.on_wait[:-1]):
                        w = mybir.InstEventSemaphore(
                            name=f"{ins.name}-hoistwait{k}",
                            ins=[],
                            outs=[],
                        )
                        w.engine = ins.engine
                        w.sync_info = mybir.SyncInfo(on_wait=[wait], on_update=[])
                        out.append(w)
                    ins.sync_info = mybir.SyncInfo(
                        on_wait=[si.on_wait[-1]], on_update=list(si.on_update)
                    )
                out.append(ins)
            bb.instructions[:] = out


_NC_CACHE = {}


def make_in_maps(inputs):
    in_maps = []
    for c in range(8):
        b, s = c // 2, c % 2
        xb = np.asarray(inputs["x"][b], dtype=np.float32)
        xp = xb if s == 0 else np.ascontiguousarray(
            np.concatenate([xb[NQ:], xb[:NQ]], axis=0)
        )
        m = {"x": xp}
        for k in INPUT_NAMES:
            m[k] = np.asarray(inputs[k], dtype=np.float32)
        in_maps.append(m)
    return in_maps


def kernel(**inputs):
    from concourse import bass_utils

    if "nc" not in _NC_CACHE:
        _NC_CACHE["nc"] = build_nc()
    nc = _NC_CACHE["nc"]
    in_maps = make_in_maps(inputs)
    res = bass_utils.run_bass_kernel_spmd(nc, in_maps, core_ids=list(range(8)))
    out = np.empty((B, N, D), np.float32)
    for c in range(8):
        b, s = c // 2, c % 2
        out[b, s * NQ:(s + 1) * NQ] = res.results[c]["out"]
    return out


if __name__ == "__main__":
    rng = np.random.default_rng(0)
    fake = {
        "x": rng.standard_normal((B, N, D), dtype=np.float32),
        "ln1_g": np.ones(D, np.float32), "ln1_b": np.zeros(D, np.float32),
        "qkv_w": (rng.standard_normal((D, 3 * D)) / np.sqrt(D)).astype(np.float32),
        "proj_w": (rng.standard_normal((D, D)) / np.sqrt(D)).astype(np.float32),
        "proj_b": np.zeros(D, np.float32),
        "ln2_g": np.ones(D, np.float32), "ln2_b": np.zeros(D, np.float32),
        "fc1_w": (rng.standard_normal((D, HID)) / np.sqrt(D)).astype(np.float32),
        "fc1_b": np.zeros(HID, np.float32),
        "fc2_w": (rng.standard_normal((HID, D)) / np.sqrt(HID)).astype(np.float32),
        "fc2_b": np.zeros(D, np.float32),
    }
    out = kernel(**fake)
    print("kernel ran, out shape", out.shape)



# revision 8
# speedup vs baseline: 1.0529x; 1.0529x over previous
"""Trainium2 Bass kernel for a ViT-style EncoderBlock.

Problem: B=4, N=2048, D=768, H=12 heads (hd=64), FFN hidden 3072, fp32.
  y = x + proj(attn(LN1(x))) ;  out = y + fc2(gelu(fc1(LN2(y))))

Sharding (8 cores, zero communication): core c handles batch b=c//2 and
query-half s=c%2 (1024 query rows).  Each core receives the full batch-b
sequence (2048 rows) with its own query rows permuted to the front, computes
K/V over all 2048 rows, attention/FFN for its 1024 rows, and returns its
[1024, 768] slice of the output.  Host reassembles.

Design notes (v2):
 - LN transposes are real bf16 matmuls against identity (full-rate, counts
   as PE activity for the HAM clock gate) instead of transpose-mode ops.
 - Softmax normalization is deferred off the PE critical path: AV matmuls
   carry a per-head ones-column at column 64+h so each head's denominator
   lands on PSUM partition 64+h; denominators accumulate in SBUF and one
   batched reciprocal + broadcast-matmul pass normalizes O at the end.
 - O is stored head-pair-packed [128, 6, NQ] so proj runs K=128 chains.
 - Softmax skips max-subtraction (scores ~N(0,1) after 1/8 scale).
"""

import sys

if "/opt/trn_rl_repo" not in sys.path:
    sys.path.insert(0, "/opt/trn_rl_repo")

import numpy as np

B, N, D = 4, 2048, 768
H, HD = 12, 64
HID = 4 * D
NQ = N // 2  # query rows per core
SCALE = HD ** -0.5
EPS = 1e-5

P = 128
DT = D // P          # 6 d-tiles
NQT = NQ // P        # 8 query tiles
NMT = N // P         # 16 kv tiles
HIDT = HID // P      # 24 hidden tiles
VW = HD + 1          # 65: V plus ones column

INPUT_NAMES = (
    "ln1_g", "ln1_b", "qkv_w", "proj_w", "proj_b",
    "ln2_g", "ln2_b", "fc1_w", "fc1_b", "fc2_w", "fc2_b",
)


def _layernorm_normalize(nc, small, work, x_sl, out_dt):
    """Row-wise LN of x_sl [128, 768] -> normalized bf16 tile (no g/b;
    those are applied post-transpose as per-partition scalars)."""
    import concourse.bass as bass
    from concourse import mybir

    st = small.tile([P, 2, 6], mybir.dt.float32, name="ln_st")
    for g in range(2):
        nc.vector.bn_stats(st[:, g, :], x_sl[:, g * 384:(g + 1) * 384])
    mv = small.tile([P, 2], mybir.dt.float32, name="ln_mv")
    nc.vector.bn_aggr(mv, st)
    ve = small.tile([P, 1], mybir.dt.float32, name="ln_ve")
    nc.vector.tensor_scalar_add(ve, mv[:, 1:2], EPS)
    sq = small.tile([P, 1], mybir.dt.float32, name="ln_sq")
    nc.scalar.sqrt(sq, ve)
    r0 = small.tile([P, 1], mybir.dt.float32, name="ln_r0")
    nc.vector.reciprocal(r0, sq)
    # one Newton step for rsqrt(ve): r = r0*(1.5 - 0.5*ve*r0^2)
    t = small.tile([P, 1], mybir.dt.float32, name="ln_t")
    nc.vector.tensor_scalar(t, r0, r0, ve, mybir.AluOpType.mult, mybir.AluOpType.mult)
    nc.vector.tensor_scalar(t, t, -0.5, 1.5, mybir.AluOpType.mult, mybir.AluOpType.add)
    r = small.tile([P, 1], mybir.dt.float32, name="ln_r")
    nc.vector.tensor_mul(r, r0, t)
    # nmr = -mu * r
    nmr = small.tile([P, 1], mybir.dt.float32, name="ln_nmr")
    nc.vector.tensor_scalar(
        nmr, mv[:, 0:1], r, -1.0, mybir.AluOpType.mult, mybir.AluOpType.mult
    )
    h = work.tile([P, D], out_dt, name="ln_h")
    nc.vector.tensor_scalar(h, x_sl, r, nmr, mybir.AluOpType.mult, mybir.AluOpType.add)
    return h


def _encoder_body(tc, out_ap, aps):
    import concourse.bass as bass
    from concourse import mybir
    from concourse.masks import make_identity

    nc = tc.nc
    f32 = mybir.dt.float32
    f32r = mybir.dt.float32r
    bf16 = mybir.dt.bfloat16
    AF = mybir.ActivationFunctionType
    OP = mybir.AluOpType

    def mm(psum, lhsT, rhs, start, stop):
        nc.tensor.matmul(psum, lhsT, rhs, start=start, stop=stop)

    x = aps["x"]

    # ---------------- constants ----------------
    consts = tc.alloc_tile_pool(name="consts", bufs=1)
    ident = consts.tile([P, P], f32, name="ident")
    make_identity(nc, ident)
    g1c = consts.tile([P, DT], f32, name="g1c")
    b1c = consts.tile([P, DT], f32, name="b1c")
    g2c = consts.tile([P, DT], f32, name="g2c")
    b2c = consts.tile([P, DT], f32, name="b2c")
    f1bc = consts.tile([P, HIDT], f32, name="f1bc")
    nc.gpsimd.dma_start(out=g1c, in_=aps["ln1_g"].rearrange("(t p) -> p t", p=P))
    nc.gpsimd.dma_start(out=b1c, in_=aps["ln1_b"].rearrange("(t p) -> p t", p=P))
    nc.gpsimd.dma_start(out=g2c, in_=aps["ln2_g"].rearrange("(t p) -> p t", p=P))
    nc.gpsimd.dma_start(out=b2c, in_=aps["ln2_b"].rearrange("(t p) -> p t", p=P))
    nc.gpsimd.dma_start(out=f1bc, in_=aps["fc1_b"].rearrange("(t p) -> p t", p=P))
    # pjbf = proj_b + fc2_b broadcast to all partitions (both residual biases)
    pjb = consts.tile([P, D], f32, name="pjb")
    f2b = consts.tile([P, D], f32, name="f2b")
    nc.gpsimd.dma_start(out=pjb, in_=aps["proj_b"].partition_broadcast(P))
    nc.gpsimd.dma_start(out=f2b, in_=aps["fc2_b"].partition_broadcast(P))
    pjbf = consts.tile([P, D], f32, name="pjbf")
    nc.vector.tensor_add(pjbf, pjb, f2b)
    ones_f = consts.tile([P, HD], f32, name="ones_f")
    nc.vector.memset(ones_f, 1.0)
    # f32r tiles cannot be memset directly — stage through f32.
    ones64 = consts.tile([1, HD], f32r, name="ones64")
    nc.vector.tensor_copy(ones64, ones_f[0:1, :])
    identb = consts.tile([P, P], bf16, name="identb")
    nc.vector.tensor_copy(identb, ident)

    # FFN weights: allocate early so the cast-DMAs prefetch during the
    # attention phase (DMA engines are idle there).
    f1wp = tc.alloc_tile_pool(name="f1w", bufs=1)
    w1a = f1wp.tile([P, DT, HID], bf16, name="w1a")
    nc.gpsimd.dma_start(out=w1a, in_=aps["fc1_w"].rearrange("(t p) c -> p t c", p=P))
    f2wp = tc.alloc_tile_pool(name="f2w", bufs=1)
    w2a = f2wp.tile([P, HIDT, D], bf16, name="w2a")
    nc.gpsimd.dma_start(out=w2a, in_=aps["fc2_w"].rearrange("(j p) d -> p j d", p=P))

    # ---------------- phase 1+2 : LN1 -> hT ; QKV ----------------
    qkv = tc.alloc_tile_pool(name="qkv", bufs=1)
    qT = qkv.tile([P, DT, NQ], bf16, name="qT")       # [qcol, nq] 6x[128,1024]
    kT = qkv.tile([P, DT, N], bf16, name="kT")        # [kcol, m]  6x[128,2048]
    V4 = qkv.tile([P, NMT, H, VW], bf16, name="V4")   # [m, mt, h, 76]

    hTp = tc.alloc_tile_pool(name="hTp", bufs=1)
    hT = hTp.tile([P, DT, N], bf16, name="hT")        # LN1(x)^T

    with tc.tile_pool(name="p1work", bufs=6) as work, \
         tc.tile_pool(name="p1small", bufs=8) as small, \
         tc.tile_pool(name="p1psum", bufs=4, space="PSUM") as psT:
        for i in range(NMT):
            xt = work.tile([P, D], f32, name="xt")
            nc.sync.dma_start(out=xt, in_=x[i * P:(i + 1) * P, :])
            h = _layernorm_normalize(nc, small, work, xt, bf16)
            for t in range(DT):
                ps = psT.tile([P, P], f32, name="trps")
                # real bf16 matmul transpose: ps = h_slice.T @ I
                mm(ps, h[:, t * P:(t + 1) * P], identb, True, True)
                nc.scalar.activation(
                    hT[:, t, i * P:(i + 1) * P], ps, AF.Identity,
                    bias=b1c[:, t:t + 1], scale=g1c[:, t:t + 1],
                )

    # QKV projections (weights resident one of q/k/v at a time)
    with tc.tile_pool(name="wqkv", bufs=1) as wpool, \
         tc.tile_pool(name="p2psum", bufs=2, space="PSUM") as psQ:
        # --- Q: qT[c, nq] = Wq[d, c].T @ hT[d, nq] ---
        wq = wpool.tile([P, DT, D], bf16, name="wq", tag="w")
        nc.gpsimd.dma_start(
            out=wq, in_=aps["qkv_w"][:, 0:D].rearrange("(t p) c -> p t c", p=P)
        )
        for c in range(DT):
            for ch in range(2):
                ps = psQ.tile([P, 512], f32, name="qps", tag="ps")
                for t in range(DT):
                    mm(ps, wq[:, t, c * P:(c + 1) * P],
                       hT[:, t, ch * 512:(ch + 1) * 512], t == 0, t == DT - 1)
                nc.scalar.copy(qT[:, c, ch * 512:(ch + 1) * 512], ps)
        # --- K ---
        wk = wpool.tile([P, DT, D], bf16, name="wk", tag="w")
        nc.gpsimd.dma_start(
            out=wk, in_=aps["qkv_w"][:, D:2 * D].rearrange("(t p) c -> p t c", p=P)
        )
        for c in range(DT):
            for ch in range(4):
                ps = psQ.tile([P, 512], f32, name="kps", tag="ps")
                for t in range(DT):
                    mm(ps, wk[:, t, c * P:(c + 1) * P],
                       hT[:, t, ch * 512:(ch + 1) * 512], t == 0, t == DT - 1)
                nc.scalar.copy(kT[:, c, ch * 512:(ch + 1) * 512], ps)
        # --- V (natural layout [m, vcol]) + per-head denominator columns ---
        wv = wpool.tile([P, DT, D], bf16, name="wv", tag="w")
        nc.gpsimd.dma_start(
            out=wv, in_=aps["qkv_w"][:, 2 * D:3 * D].rearrange("(t p) c -> p t c", p=P)
        )
        for mt in range(NMT):
            psv = psQ.tile([P, D], f32, name="vps", tag="psv")
            for t in range(DT):
                mm(psv[:, 0:512], hT[:, t, mt * P:(mt + 1) * P],
                   wv[:, t, 0:512], t == 0, t == DT - 1)
            for t in range(DT):
                mm(psv[:, 512:768], hT[:, t, mt * P:(mt + 1) * P],
                   wv[:, t, 512:768], t == 0, t == DT - 1)
            nc.scalar.copy(
                V4[:, mt, 0:8, 0:HD], psv[:, 0:512].rearrange("p (a b) -> p a b", a=8)
            )
            nc.scalar.copy(
                V4[:, mt, 8:12, 0:HD],
                psv[:, 512:768].rearrange("p (a b) -> p a b", a=4),
            )
            nc.vector.tensor_copy(
                V4[:, mt, :, HD:VW], ones_f[:, 0:H].rearrange("p (a b) -> p a b", b=1)
            )

    hTp.release()

    # ---------------- phase 3 : attention ----------------
    # OT2: head-pair packed normalized O^T: partitions 0:64 = head 2j,
    # partitions 64:128 = head 2j+1 (so proj runs K=128 chains).
    # Normalization is software-pipelined: chunk c's reciprocal runs on DVE
    # while chunk c+1's matmuls stream; the broadcast matmuls + multiplies
    # for chunk c are emitted in the middle of chunk c+1's mt loop so the
    # PE never waits on the DVE reciprocal.
    otp = tc.alloc_tile_pool(name="otp", bufs=1, side="right")
    OT2 = otp.tile([P, H // 2, NQ], bf16, name="OT2")

    with tc.tile_pool(name="a_es", bufs=6) as esp, \
         tc.tile_pool(name="a_small", bufs=4) as asmall, \
         tc.tile_pool(name="a_psS", bufs=2, space="PSUM") as psS, \
         tc.tile_pool(name="a_psO", bufs=2, space="PSUM") as psO, \
         tc.tile_pool(name="a_psR", bufs=2, space="PSUM") as psR:

        def emit_norm(st):
            j, cs, posbA, posbB, rdA, rdB = st
            rbA = psR.tile([HD, 512], f32, name="rbA", tag="rb")
            mm(rbA, ones64, rdA, True, True)
            nc.vector.tensor_tensor(OT2[0:HD, j, cs], posbA[0:HD, :], rbA, OP.mult)
            rbB = psR.tile([HD, 512], f32, name="rbB", tag="rb")
            mm(rbB, ones64, rdB, True, True)
            nc.vector.tensor_tensor(OT2[HD:P, j, cs], posbB[0:HD, :], rbB, OP.mult)

        pending = None
        for j in range(H // 2):
            hA, hB = 2 * j, 2 * j + 1
            kTa, kTb = kT[0:HD, j, :], kT[HD:P, j, :]
            qTa, qTb = qT[0:HD, j, :], qT[HD:P, j, :]
            for ch in range(2):
                cs = slice(ch * 512, (ch + 1) * 512)
                poA = psO.tile([VW, 512], f32, name="poA", tag="po")
                poB = psO.tile([VW, 512], f32, name="poB", tag="po")
                for mt in range(NMT):
                    msl = slice(mt * P, (mt + 1) * P)
                    ps = psS.tile([P, 1024], f32, name="sps")
                    # two heads on the two 64-row halves of the PE array
                    mm(ps[:, 0:512], kTa[:, msl], qTa[:, cs], True, True)
                    mm(ps[:, 512:1024], kTb[:, msl], qTb[:, cs], True, True)
                    es = esp.tile([P, 1024], bf16, name="es")
                    nc.scalar.activation(es, ps, AF.Exp, scale=SCALE)
                    mm(poA, V4[:, mt, hA, :], es[:, 0:512], mt == 0, mt == NMT - 1)
                    mm(poB, V4[:, mt, hB, :], es[:, 512:1024], mt == 0, mt == NMT - 1)
                    if mt == 5 and pending is not None:
                        emit_norm(pending)
                        pending = None
                # drain to SBUF + reciprocal of the denominator row (DVE)
                posbA = asmall.tile([VW, 512], f32, name="posbA", tag="posbA", bufs=2)
                nc.vector.tensor_copy(posbA, poA)
                posbB = asmall.tile([VW, 512], f32, name="posbB", tag="posbB", bufs=2)
                nc.vector.tensor_copy(posbB, poB)
                rdA = asmall.tile([1, 512], f32r, name="rdA", tag="rdA", bufs=2)
                rdB = asmall.tile([1, 512], f32r, name="rdB", tag="rdB", bufs=2)
                with nc.allow_low_precision(reason="softmax denom recip"):
                    nc.vector.reciprocal(rdA, posbA[HD:VW, :])
                    nc.vector.reciprocal(rdB, posbB[HD:VW, :])
                pending = (j, cs, posbA, posbB, rdA, rdB)
        emit_norm(pending)

    qkv.release()

    # ---------------- phase 4 : proj + residual + LN2 -> x2T ----------------
    res1p = tc.alloc_tile_pool(name="res1p", bufs=1)
    res1 = res1p.tile([P, NQT, D], f32, name="res1")
    x2Tp = tc.alloc_tile_pool(name="x2Tp", bufs=1)
    x2T = x2Tp.tile([P, DT, NQ], bf16, name="x2T")

    with tc.tile_pool(name="pjw", bufs=1) as pjwp, \
         tc.tile_pool(name="p4work", bufs=6) as work4, \
         tc.tile_pool(name="p4small", bufs=8) as small4, \
         tc.tile_pool(name="p4psum", bufs=2, space="PSUM") as psP, \
         tc.tile_pool(name="p4psT", bufs=3, space="PSUM") as psT4:
        # proj weights packed by head pair: partition p of pair j is
        # proj_w row 128*j + p (heads 2j and 2j+1 are contiguous).
        pjw = pjwp.tile([P, H // 2, D], bf16, name="pjw")
        nc.gpsimd.dma_start(out=pjw, in_=aps["proj_w"].rearrange("(j p) d -> p j d", p=P))
        for i in range(NQT):
            isl = slice(i * P, (i + 1) * P)
            psp = psP.tile([P, D], f32, name="psp")
            for j in range(H // 2):
                mm(psp[:, 0:512], OT2[:, j, isl], pjw[:, j, 0:512],
                   j == 0, j == H // 2 - 1)
            for j in range(H // 2):
                mm(psp[:, 512:768], OT2[:, j, isl], pjw[:, j, 512:768],
                   j == 0, j == H // 2 - 1)
            xr = work4.tile([P, D], f32, name="xr")
            nc.sync.dma_start(out=xr, in_=x[i * P:(i + 1) * P, :])
            r1 = res1[:, i, :]
            nc.vector.tensor_add(r1, psp, xr)
            nc.vector.tensor_add(r1, r1, pjbf)
            h2 = _layernorm_normalize(nc, small4, work4, r1, bf16)
            for t in range(DT):
                ps = psT4.tile([P, P], f32, name="trps4")
                mm(ps, h2[:, t * P:(t + 1) * P], identb, True, True)
                nc.scalar.activation(
                    x2T[:, t, i * P:(i + 1) * P], ps, AF.Identity,
                    bias=b2c[:, t:t + 1], scale=g2c[:, t:t + 1],
                )
        otp.release()

    # ---------------- phase 5 : FFN fused per nq-chunk ----------------
    h1p = tc.alloc_tile_pool(name="h1p", bufs=1, side="right")
    with tc.tile_pool(name="p5psum", bufs=2, space="PSUM") as psF, \
         tc.tile_pool(name="p6psum", bufs=2, space="PSUM") as ps2:
      for ch in range(2):
        h1 = h1p.tile([P, HIDT, 512], bf16, name="h1", tag="h1")
        for hc in range(HIDT):
            psf = psF.tile([P, 512], f32, name="psf")
            for t in range(DT):
                mm(psf, w1a[:, t, hc * P:(hc + 1) * P],
                   x2T[:, t, ch * 512:(ch + 1) * 512], t == 0, t == DT - 1)
            nc.scalar.activation(
                h1[:, hc, :], psf, AF.Gelu, bias=f1bc[:, hc:hc + 1]
            )
        for i2 in range(4):
            i = ch * 4 + i2
            psq = ps2.tile([P, D], f32, name="psq")
            for j in range(HIDT):
                mm(psq[:, 0:512], h1[:, j, i2 * P:(i2 + 1) * P],
                   w2a[:, j, 0:512], j == 0, j == HIDT - 1)
            for j in range(HIDT):
                mm(psq[:, 512:768], h1[:, j, i2 * P:(i2 + 1) * P],
                   w2a[:, j, 512:768], j == 0, j == HIDT - 1)
            nc.vector.tensor_add(res1[:, i, :], res1[:, i, :], psq)
            nc.sync.dma_start(out=out_ap[i * P:(i + 1) * P, :], in_=res1[:, i, :])
    x2Tp.release()
    h1p.release()
    res1p.release()
    f2wp.release()
    f1wp.release()
    consts.release()


def build_nc(hoist_waits=True):
    import concourse.bass as bass
    import concourse.tile as tile
    from concourse import mybir

    f32 = mybir.dt.float32
    nc = bass.Bass("TRN2", target_bir_lowering=False, debug=False)
    aps = {"x": nc.dram_tensor("x", [N, D], f32, kind="ExternalInput").ap()}
    shapes = {
        "ln1_g": [D], "ln1_b": [D], "qkv_w": [D, 3 * D],
        "proj_w": [D, D], "proj_b": [D], "ln2_g": [D], "ln2_b": [D],
        "fc1_w": [D, HID], "fc1_b": [HID], "fc2_w": [HID, D], "fc2_b": [D],
    }
    for name in INPUT_NAMES:
        aps[name] = nc.dram_tensor(name, shapes[name], f32, kind="ExternalInput").ap()
    out_ap = nc.dram_tensor("out", [NQ, D], f32, kind="ExternalOutput").ap()
    with tile.TileContext(nc) as tc:
        _encoder_body(tc, out_ap, aps)
    if hoist_waits:
        _hoist_matmul_waits(nc)
    return nc


def _hoist_matmul_waits(nc):
    """walrus's LW-path matmuls (transpose / fp32 / f32r self-loading) accept
    only one embedded sync-wait.  Tile can attach two (one per producer
    engine).  Hoist all-but-one onto a standalone InstEventSemaphore placed
    just before the matmul in the same engine stream."""
    from concourse import mybir

    skip = (
        mybir.InstEventSemaphore,
        mybir.InstUnconditionalBranch,
    )
    for f in nc.m.functions:
        for bb in f.blocks:
            out = []
            for ins in bb.instructions:
                si = getattr(ins, "sync_info", None)
                if (
                    si is not None
                    and si.on_wait
                    and len(si.on_wait) > 1
                    and not isinstance(ins, skip)
                ):
                    for k, wait in enumerate(si.on_wait[:-1]):
                        w = mybir.InstEventSemaphore(
                            name=f"{ins.name}-hoistwait{k}",
                            ins=[],
                            outs=[],
                        )
                        w.engine = ins.engine
                        w.sync_info = mybir.SyncInfo(on_wait=[wait], on_update=[])
                        out.append(w)
                    ins.sync_info = mybir.SyncInfo(
                        on_wait=[si.on_wait[-1]], on_update=list(si.on_update)
                    )
                out.append(ins)
            bb.instructions[:] = out


_NC_CACHE = {}


def make_in_maps(inputs):
    in_maps = []
    for c in range(8):
        b, s = c // 2, c % 2
        xb = np.asarray(inputs["x"][b], dtype=np.float32)
        xp = xb if s == 0 else np.ascontiguousarray(
            np.concatenate([xb[NQ:], xb[:NQ]], axis=0)
        )
        m = {"x": xp}
        for k in INPUT_NAMES:
            m[k] = np.asarray(inputs[k], dtype=np.float32)
        in_maps.append(m)
    return in_maps


def kernel(**inputs):
    from concourse import bass_utils

    if "nc" not in _NC_CACHE:
        _NC_CACHE["nc"] = build_nc()
    nc = _NC_CACHE["nc"]
    in_maps = make_in_maps(inputs)
    res = bass_utils.run_bass_kernel_spmd(nc, in_maps, core_ids=list(range(8)))
    out = np.empty((B, N, D), np.float32)
    for c in range(8):
        b, s = c // 2, c % 2
        out[b, s * NQ:(s + 1) * NQ] = res.results[c]["out"]
    return out


if __name__ == "__main__":
    rng = np.random.default_rng(0)
    fake = {
        "x": rng.standard_normal((B, N, D), dtype=np.float32),
        "ln1_g": np.ones(D, np.float32), "ln1_b": np.zeros(D, np.float32),
        "qkv_w": (rng.standard_normal((D, 3 * D)) / np.sqrt(D)).astype(np.float32),
        "proj_w": (rng.standard_normal((D, D)) / np.sqrt(D)).astype(np.float32),
        "proj_b": np.zeros(D, np.float32),
        "ln2_g": np.ones(D, np.float32), "ln2_b": np.zeros(D, np.float32),
        "fc1_w": (rng.standard_normal((D, HID)) / np.sqrt(D)).astype(np.float32),
        "fc1_b": np.zeros(HID, np.float32),
        "fc2_w": (rng.standard_normal((HID, D)) / np.sqrt(HID)).astype(np.float32),
        "fc2_b": np.zeros(D, np.float32),
    }
    out = kernel(**fake)
    print("kernel ran, out shape", out.shape)


# revision 17
# speedup vs baseline: 1.1689x; 1.1101x over previous
"""Trainium2 Bass kernel for a ViT-style EncoderBlock.

Problem: B=4, N=2048, D=768, H=12 heads (hd=64), FFN hidden 3072, fp32.
  y = x + proj(attn(LN1(x))) ;  out = y + fc2(gelu(fc1(LN2(y))))

Sharding (8 cores, zero communication): core c handles batch b=c//2 and
query-half s=c%2 (1024 query rows).  Each core receives the full batch-b
sequence (2048 rows) with its own query rows permuted to the front, computes
K/V over all 2048 rows, attention/FFN for its 1024 rows, and returns its
[1024, 768] slice of the output.  Host reassembles.

Design notes (v2):
 - LN transposes are real bf16 matmuls against identity (full-rate, counts
   as PE activity for the HAM clock gate) instead of transpose-mode ops.
 - Softmax normalization is deferred off the PE critical path: AV matmuls
   carry a per-head ones-column at column 64+h so each head's denominator
   lands on PSUM partition 64+h; denominators accumulate in SBUF and one
   batched reciprocal + broadcast-matmul pass normalizes O at the end.
 - O is stored head-pair-packed [128, 6, NQ] so proj runs K=128 chains.
 - Softmax skips max-subtraction (scores ~N(0,1) after 1/8 scale).
"""

import sys

if "/opt/trn_rl_repo" not in sys.path:
    sys.path.insert(0, "/opt/trn_rl_repo")

import numpy as np

B, N, D = 4, 2048, 768
H, HD = 12, 64
HID = 4 * D
NQ = N // 2  # query rows per core
SCALE = HD ** -0.5
EPS = 1e-5

P = 128
DT = D // P          # 6 d-tiles
NQT = NQ // P        # 8 query tiles
NMT = N // P         # 16 kv tiles
HIDT = HID // P      # 24 hidden tiles
VW = HD + 1          # 65: V plus ones column

INPUT_NAMES = (
    "ln1_g", "ln1_b", "qkv_w", "proj_w", "proj_b",
    "ln2_g", "ln2_b", "fc1_w", "fc1_b", "fc2_w", "fc2_b",
)


def _layernorm_normalize(nc, small, work, x_sl, out_dt):
    """Row-wise LN of x_sl [128, 768] -> normalized bf16 tile (no g/b;
    those are applied post-transpose as per-partition scalars)."""
    import concourse.bass as bass
    from concourse import mybir

    st = small.tile([P, 2, 6], mybir.dt.float32, name="ln_st")
    for g in range(2):
        nc.vector.bn_stats(st[:, g, :], x_sl[:, g * 384:(g + 1) * 384])
    mv = small.tile([P, 2], mybir.dt.float32, name="ln_mv")
    nc.vector.bn_aggr(mv, st)
    ve = small.tile([P, 1], mybir.dt.float32, name="ln_ve")
    nc.vector.tensor_scalar_add(ve, mv[:, 1:2], EPS)
    sq = small.tile([P, 1], mybir.dt.float32, name="ln_sq")
    nc.scalar.sqrt(sq, ve)
    r0 = small.tile([P, 1], mybir.dt.float32, name="ln_r0")
    nc.vector.reciprocal(r0, sq)
    # one Newton step for rsqrt(ve): r = r0*(1.5 - 0.5*ve*r0^2)
    t = small.tile([P, 1], mybir.dt.float32, name="ln_t")
    nc.vector.tensor_scalar(t, r0, r0, ve, mybir.AluOpType.mult, mybir.AluOpType.mult)
    nc.vector.tensor_scalar(t, t, -0.5, 1.5, mybir.AluOpType.mult, mybir.AluOpType.add)
    r = small.tile([P, 1], mybir.dt.float32, name="ln_r")
    nc.vector.tensor_mul(r, r0, t)
    # nmr = -mu * r
    nmr = small.tile([P, 1], mybir.dt.float32, name="ln_nmr")
    nc.vector.tensor_scalar(
        nmr, mv[:, 0:1], r, -1.0, mybir.AluOpType.mult, mybir.AluOpType.mult
    )
    h = work.tile([P, D], out_dt, name="ln_h")
    nc.vector.tensor_scalar(h, x_sl, r, nmr, mybir.AluOpType.mult, mybir.AluOpType.add)
    return h


def _encoder_body(tc, out_ap, aps):
    import concourse.bass as bass
    from concourse import mybir
    from concourse.masks import make_identity

    nc = tc.nc
    f32 = mybir.dt.float32
    f32r = mybir.dt.float32r
    bf16 = mybir.dt.bfloat16
    AF = mybir.ActivationFunctionType
    OP = mybir.AluOpType

    def mm(psum, lhsT, rhs, start, stop):
        nc.tensor.matmul(psum, lhsT, rhs, start=start, stop=stop)

    x = aps["x"]

    # ---------------- constants ----------------
    consts = tc.alloc_tile_pool(name="consts", bufs=1)
    ident = consts.tile([P, P], f32, name="ident")
    make_identity(nc, ident)
    g1c = consts.tile([P, DT], f32, name="g1c")
    b1c = consts.tile([P, DT], f32, name="b1c")
    g2c = consts.tile([P, DT], f32, name="g2c")
    b2c = consts.tile([P, DT], f32, name="b2c")
    f1bc = consts.tile([P, HIDT], f32, name="f1bc")
    nc.gpsimd.dma_start(out=g1c, in_=aps["ln1_g"].rearrange("(t p) -> p t", p=P))
    nc.gpsimd.dma_start(out=b1c, in_=aps["ln1_b"].rearrange("(t p) -> p t", p=P))
    nc.gpsimd.dma_start(out=g2c, in_=aps["ln2_g"].rearrange("(t p) -> p t", p=P))
    nc.gpsimd.dma_start(out=b2c, in_=aps["ln2_b"].rearrange("(t p) -> p t", p=P))
    nc.gpsimd.dma_start(out=f1bc, in_=aps["fc1_b"].rearrange("(t p) -> p t", p=P))
    # pjbf = proj_b + fc2_b broadcast to all partitions (both residual biases)
    pjb = consts.tile([P, D], f32, name="pjb")
    f2b = consts.tile([P, D], f32, name="f2b")
    nc.gpsimd.dma_start(out=pjb, in_=aps["proj_b"].partition_broadcast(P))
    nc.gpsimd.dma_start(out=f2b, in_=aps["fc2_b"].partition_broadcast(P))
    pjbf = consts.tile([P, D], f32, name="pjbf")
    nc.vector.tensor_add(pjbf, pjb, f2b)
    ones_f = consts.tile([P, HD], f32, name="ones_f")
    nc.vector.memset(ones_f, 1.0)
    # f32r tiles cannot be memset directly — stage through f32.
    ones64 = consts.tile([1, HD], f32r, name="ones64")
    nc.vector.tensor_copy(ones64, ones_f[0:1, :])
    identb = consts.tile([P, P], bf16, name="identb")
    nc.vector.tensor_copy(identb, ident)

    # QKV weights: gpsimd queue, in consumption order (V phase runs first).
    wkq = tc.alloc_tile_pool(name="wkq", bufs=1)
    wv = wkq.tile([P, DT, D], bf16, name="wv")
    wk = wkq.tile([P, DT, D], bf16, name="wk")
    wq = wkq.tile([P, DT, D], bf16, name="wq")
    nc.gpsimd.dma_start(
        out=wv, in_=aps["qkv_w"][:, 2 * D:3 * D].rearrange("(t p) c -> p t c", p=P)
    )
    nc.gpsimd.dma_start(
        out=wk, in_=aps["qkv_w"][:, D:2 * D].rearrange("(t p) c -> p t c", p=P)
    )
    nc.gpsimd.dma_start(
        out=wq, in_=aps["qkv_w"][:, 0:D].rearrange("(t p) c -> p t c", p=P)
    )

    # ---------------- phase 1 : LN1 -> hT ----------------
    qkv = tc.alloc_tile_pool(name="qkv", bufs=1)
    qT = qkv.tile([P, DT, NQ], bf16, name="qT")       # [qcol, nq] 6x[128,1024]
    kT = qkv.tile([P, DT, N], bf16, name="kT")        # [kcol, m]  6x[128,2048]
    V4 = qkv.tile([P, NMT, H, VW], bf16, name="V4")   # [m, mt, h, 65]

    hTp = tc.alloc_tile_pool(name="hTp", bufs=1)
    hT = hTp.tile([P, DT, N], bf16, name="hT")        # LN1(x)^T

    with tc.tile_pool(name="p1work", bufs=6) as work, \
         tc.tile_pool(name="p1small", bufs=8) as small, \
         tc.tile_pool(name="p1psum", bufs=4, space="PSUM") as psT:
        for i in range(NMT):
            xt = work.tile([P, D], f32, name="xt")
            nc.sync.dma_start(out=xt, in_=x[i * P:(i + 1) * P, :])
            h = _layernorm_normalize(nc, small, work, xt, bf16)
            for t in range(DT):
                ps = psT.tile([P, P], f32, name="trps")
                # real bf16 matmul transpose: ps = h_slice.T @ I
                mm(ps, h[:, t * P:(t + 1) * P], identb, True, True)
                nc.scalar.activation(
                    hT[:, t, i * P:(i + 1) * P], ps, AF.Identity,
                    bias=b1c[:, t:t + 1], scale=g1c[:, t:t + 1],
                )

    # ---------------- phase 2a : V (needed by all attention heads) --------
    with tc.tile_pool(name="p2psum", bufs=2, space="PSUM") as psV:
        for mt in range(NMT):
            psv = psV.tile([P, D], f32, name="vps")
            for t in range(DT):
                mm(psv[:, 0:512], hT[:, t, mt * P:(mt + 1) * P],
                   wv[:, t, 0:512], t == 0, t == DT - 1)
            for t in range(DT):
                mm(psv[:, 512:768], hT[:, t, mt * P:(mt + 1) * P],
                   wv[:, t, 512:768], t == 0, t == DT - 1)
            nc.scalar.copy(
                V4[:, mt, 0:8, 0:HD], psv[:, 0:512].rearrange("p (a b) -> p a b", a=8)
            )
            nc.scalar.copy(
                V4[:, mt, 8:12, 0:HD],
                psv[:, 512:768].rearrange("p (a b) -> p a b", a=4),
            )
            nc.vector.tensor_copy(
                V4[:, mt, :, HD:VW], ones_f[:, 0:H].rearrange("p (a b) -> p a b", b=1)
            )

    # ---------------- phase 3 : attention ----------------
    # OT2: head-pair packed normalized O^T: partitions 0:64 = head 2j,
    # partitions 64:128 = head 2j+1 (so proj runs K=128 chains).
    # Normalization is software-pipelined: chunk c's reciprocal runs on DVE
    # while chunk c+1's matmuls stream; the broadcast matmuls + multiplies
    # for chunk c are emitted in the middle of chunk c+1's mt loop so the
    # PE never waits on the DVE reciprocal.
    otp = tc.alloc_tile_pool(name="otp", bufs=1, side="right")
    OT2 = otp.tile([P, H // 2, NQ], bf16, name="OT2")

    with tc.tile_pool(name="a_es", bufs=6) as esp, \
         tc.tile_pool(name="a_small", bufs=4) as asmall, \
         tc.tile_pool(name="a_psS", bufs=2, space="PSUM") as psS, \
         tc.tile_pool(name="a_psO", bufs=2, space="PSUM") as psO, \
         tc.tile_pool(name="a_psQ", bufs=2, space="PSUM") as psQ:

        def emit_kq_chain(kind, c, idx):
            """One 512-wide K or Q projection chain for head-column c."""
            ps = psQ.tile([P, 512], f32, name="cq", tag="cq")
            w_ = wk if kind == "K" else wq
            for t in range(DT):
                mm(ps, w_[:, t, c * P:(c + 1) * P],
                   hT[:, t, idx * 512:(idx + 1) * 512], t == 0, t == DT - 1)
            if kind == "K":
                nc.vector.tensor_copy(kT[:, c, idx * 512:(idx + 1) * 512], ps)
            else:
                nc.vector.tensor_copy(qT[:, c, idx * 512:(idx + 1) * 512], ps)

        def emit_norm(st):
            j, cs, posbA, posbB, rdA, rdB = st
            rbA = psQ.tile([HD, 512], f32, name="rbA", tag="cq")
            mm(rbA, ones64, rdA, True, True)
            nc.vector.tensor_tensor(OT2[0:HD, j, cs], posbA[0:HD, :], rbA, OP.mult)
            rbB = psQ.tile([HD, 512], f32, name="rbB", tag="cq")
            mm(rbB, ones64, rdB, True, True)
            nc.vector.tensor_tensor(OT2[HD:P, j, cs], posbB[0:HD, :], rbB, OP.mult)

        # head-column 0 of K and Q (the j=0 prerequisites)
        for idx in range(4):
            emit_kq_chain("K", 0, idx)
        for idx in range(2):
            emit_kq_chain("Q", 0, idx)

        pending = None
        for j in range(H // 2):
            hA, hB = 2 * j, 2 * j + 1
            kTa, kTb = kT[0:HD, j, :], kT[HD:P, j, :]
            qTa, qTb = qT[0:HD, j, :], qT[HD:P, j, :]
            # K/Q chains for head-column j+1, spread across this j's loops
            chains = []
            if j + 1 < H // 2:
                chains = [("K", j + 1, 0), ("K", j + 1, 1), ("K", j + 1, 2),
                          ("K", j + 1, 3), ("Q", j + 1, 0), ("Q", j + 1, 1)]
            for ch in range(2):
                cs = slice(ch * 512, (ch + 1) * 512)
                poA = psO.tile([VW, 512], f32, name="poA", tag="po")
                poB = psO.tile([VW, 512], f32, name="poB", tag="po")
                for mt in range(NMT):
                    msl = slice(mt * P, (mt + 1) * P)
                    ps = psS.tile([P, 1024], f32, name="sps")
                    # two heads on the two 64-row halves of the PE array
                    mm(ps[:, 0:512], kTa[:, msl], qTa[:, cs], True, True)
                    mm(ps[:, 512:1024], kTb[:, msl], qTb[:, cs], True, True)
                    es = esp.tile([P, 1024], bf16, name="es")
                    nc.scalar.activation(es, ps, AF.Exp, scale=SCALE)
                    mm(poA, V4[:, mt, hA, :], es[:, 0:512], mt == 0, mt == NMT - 1)
                    mm(poB, V4[:, mt, hB, :], es[:, 512:1024], mt == 0, mt == NMT - 1)
                    if mt == 5 and pending is not None:
                        emit_norm(pending)
                        pending = None
                    if mt in (3, 8, 13) and chains:
                        emit_kq_chain(*chains.pop(0))
                # drain to SBUF + reciprocal of the denominator row (DVE)
                posbA = asmall.tile([VW, 512], f32, name="posbA", tag="posbA", bufs=2)
                nc.vector.tensor_copy(posbA, poA)
                posbB = asmall.tile([VW, 512], f32, name="posbB", tag="posbB", bufs=2)
                nc.vector.tensor_copy(posbB, poB)
                rdA = asmall.tile([1, 512], f32r, name="rdA", tag="rdA", bufs=2)
                rdB = asmall.tile([1, 512], f32r, name="rdB", tag="rdB", bufs=2)
                with nc.allow_low_precision(reason="softmax denom recip"):
                    nc.vector.reciprocal(rdA, posbA[HD:VW, :])
                    nc.vector.reciprocal(rdB, posbB[HD:VW, :])
                pending = (j, cs, posbA, posbB, rdA, rdB)
        emit_norm(pending)

    hTp.release()
    qkv.release()
    wkq.release()

    # ---------------- phase 4 : proj + residual + LN2 -> x2T ----------------
    res1p = tc.alloc_tile_pool(name="res1p", bufs=1)
    res1 = res1p.tile([P, NQT, D], f32, name="res1")
    x2Tp = tc.alloc_tile_pool(name="x2Tp", bufs=1)
    x2T = x2Tp.tile([P, DT, NQ], bf16, name="x2T")
    f1wp = tc.alloc_tile_pool(name="f1w", bufs=1)
    w1a = f1wp.tile([P, DT, HID], bf16, name="w1a")
    f2wp = tc.alloc_tile_pool(name="f2w", bufs=1)
    w2a = f2wp.tile([P, HIDT, D], bf16, name="w2a")

    with tc.tile_pool(name="pjw", bufs=1) as pjwp, \
         tc.tile_pool(name="p4work", bufs=6) as work4, \
         tc.tile_pool(name="p4small", bufs=8) as small4, \
         tc.tile_pool(name="p4psum", bufs=2, space="PSUM") as psP, \
         tc.tile_pool(name="p4psT", bufs=3, space="PSUM") as psT4:
        # proj weights packed by head pair: partition p of pair j is
        # proj_w row 128*j + p (heads 2j and 2j+1 are contiguous).
        pjw = pjwp.tile([P, H // 2, D], bf16, name="pjw")
        nc.gpsimd.dma_start(out=pjw, in_=aps["proj_w"].rearrange("(j p) d -> p j d", p=P))
        # FFN weight DMAs: gpsimd queue behind pjw so proj weights arrive
        # first (pools allocated outside this block for LIFO order).
        nc.gpsimd.dma_start(out=w1a, in_=aps["fc1_w"].rearrange("(t p) c -> p t c", p=P))
        nc.gpsimd.dma_start(out=w2a, in_=aps["fc2_w"].rearrange("(j p) d -> p j d", p=P))
        for i in range(NQT):
            isl = slice(i * P, (i + 1) * P)
            psp = psP.tile([P, D], f32, name="psp")
            for j in range(H // 2):
                mm(psp[:, 0:512], OT2[:, j, isl], pjw[:, j, 0:512],
                   j == 0, j == H // 2 - 1)
            for j in range(H // 2):
                mm(psp[:, 512:768], OT2[:, j, isl], pjw[:, j, 512:768],
                   j == 0, j == H // 2 - 1)
            xr = work4.tile([P, D], f32, name="xr")
            nc.sync.dma_start(out=xr, in_=x[i * P:(i + 1) * P, :])
            r1 = res1[:, i, :]
            nc.vector.tensor_add(r1, psp, xr)
            nc.vector.tensor_add(r1, r1, pjbf)
            h2 = _layernorm_normalize(nc, small4, work4, r1, bf16)
            for t in range(DT):
                ps = psT4.tile([P, P], f32, name="trps4")
                mm(ps, h2[:, t * P:(t + 1) * P], identb, True, True)
                nc.scalar.activation(
                    x2T[:, t, i * P:(i + 1) * P], ps, AF.Identity,
                    bias=b2c[:, t:t + 1], scale=g2c[:, t:t + 1],
                )
        otp.release()

    # ---------------- phase 5 : FFN fused per nq-chunk ----------------
    h1p = tc.alloc_tile_pool(name="h1p", bufs=1, side="right")
    with tc.tile_pool(name="p5psum", bufs=2, space="PSUM") as psF, \
         tc.tile_pool(name="p6psum", bufs=2, space="PSUM") as ps2:
      for ch in range(2):
        h1 = h1p.tile([P, HIDT, 512], bf16, name="h1", tag="h1")
        for hc in range(HIDT):
            psf = psF.tile([P, 512], f32, name="psf")
            for t in range(DT):
                mm(psf, w1a[:, t, hc * P:(hc + 1) * P],
                   x2T[:, t, ch * 512:(ch + 1) * 512], t == 0, t == DT - 1)
            nc.scalar.activation(
                h1[:, hc, :], psf, AF.Gelu, bias=f1bc[:, hc:hc + 1]
            )
        for i2 in range(4):
            i = ch * 4 + i2
            psq = ps2.tile([P, D], f32, name="psq")
            for j in range(HIDT):
                mm(psq[:, 0:512], h1[:, j, i2 * P:(i2 + 1) * P],
                   w2a[:, j, 0:512], j == 0, j == HIDT - 1)
            for j in range(HIDT):
                mm(psq[:, 512:768], h1[:, j, i2 * P:(i2 + 1) * P],
                   w2a[:, j, 512:768], j == 0, j == HIDT - 1)
            nc.vector.tensor_add(res1[:, i, :], res1[:, i, :], psq)
            nc.sync.dma_start(out=out_ap[i * P:(i + 1) * P, :], in_=res1[:, i, :])
    h1p.release()
    f2wp.release()
    f1wp.release()
    x2Tp.release()
    res1p.release()
    consts.release()


def build_nc(hoist_waits=True):
    import concourse.bass as bass
    import concourse.tile as tile
    from concourse import mybir

    f32 = mybir.dt.float32
    nc = bass.Bass("TRN2", target_bir_lowering=False, debug=False)
    aps = {"x": nc.dram_tensor("x", [N, D], f32, kind="ExternalInput").ap()}
    shapes = {
        "ln1_g": [D], "ln1_b": [D], "qkv_w": [D, 3 * D],
        "proj_w": [D, D], "proj_b": [D], "ln2_g": [D], "ln2_b": [D],
        "fc1_w": [D, HID], "fc1_b": [HID], "fc2_w": [HID, D], "fc2_b": [D],
    }
    for name in INPUT_NAMES:
        aps[name] = nc.dram_tensor(name, shapes[name], f32, kind="ExternalInput").ap()
    out_ap = nc.dram_tensor("out", [NQ, D], f32, kind="ExternalOutput").ap()
    with tile.TileContext(nc) as tc:
        _encoder_body(tc, out_ap, aps)
    if hoist_waits:
        _hoist_matmul_waits(nc)
    return nc


def _hoist_matmul_waits(nc):
    """walrus's LW-path matmuls (transpose / fp32 / f32r self-loading) accept
    only one embedded sync-wait.  Tile can attach two (one per producer
    engine).  Hoist all-but-one onto a standalone InstEventSemaphore placed
    just before the matmul in the same engine stream."""
    from concourse import mybir

    skip = (
        mybir.InstEventSemaphore,
        mybir.InstUnconditionalBranch,
    )
    for f in nc.m.functions:
        for bb in f.blocks:
            out = []
            for ins in bb.instructions:
                si = getattr(ins, "sync_info", None)
                if (
                    si is not None
                    and si.on_wait
                    and len(si.on_wait) > 1
                    and not isinstance(ins, skip)
                ):
                    for k, wait in enumerate(si.on_wait[:-1]):
                        w = mybir.InstEventSemaphore(
                            name=f"{ins.name}-hoistwait{k}",
                            ins=[],
                            outs=[],
                        )
                        w.engine = ins.engine
                        w.sync_info = mybir.SyncInfo(on_wait=[wait], on_update=[])
                        out.append(w)
                    ins.sync_info = mybir.SyncInfo(
                        on_wait=[si.on_wait[-1]], on_update=list(si.on_update)
                    )
                out.append(ins)
            bb.instructions[:] = out


_NC_CACHE = {}


def make_in_maps(inputs):
    in_maps = []
    for c in range(8):
        b, s = c // 2, c % 2
        xb = np.asarray(inputs["x"][b], dtype=np.float32)
        xp = xb if s == 0 else np.ascontiguousarray(
            np.concatenate([xb[NQ:], xb[:NQ]], axis=0)
        )
        m = {"x": xp}
        for k in INPUT_NAMES:
            m[k] = np.asarray(inputs[k], dtype=np.float32)
        in_maps.append(m)
    return in_maps


def kernel(**inputs):
    from concourse import bass_utils

    if "nc" not in _NC_CACHE:
        _NC_CACHE["nc"] = build_nc()
    nc = _NC_CACHE["nc"]
    in_maps = make_in_maps(inputs)
    res = bass_utils.run_bass_kernel_spmd(nc, in_maps, core_ids=list(range(8)))
    out = np.empty((B, N, D), np.float32)
    for c in range(8):
        b, s = c // 2, c % 2
        out[b, s * NQ:(s + 1) * NQ] = res.results[c]["out"]
    return out


if __name__ == "__main__":
    rng = np.random.default_rng(0)
    fake = {
        "x": rng.standard_normal((B, N, D), dtype=np.float32),
        "ln1_g": np.ones(D, np.float32), "ln1_b": np.zeros(D, np.float32),
        "qkv_w": (rng.standard_normal((D, 3 * D)) / np.sqrt(D)).astype(np.float32),
        "proj_w": (rng.standard_normal((D, D)) / np.sqrt(D)).astype(np.float32),
        "proj_b": np.zeros(D, np.float32),
        "ln2_g": np.ones(D, np.float32), "ln2_b": np.zeros(D, np.float32),
        "fc1_w": (rng.standard_normal((D, HID)) / np.sqrt(D)).astype(np.float32),
        "fc1_b": np.zeros(HID, np.float32),
        "fc2_w": (rng.standard_normal((HID, D)) / np.sqrt(HID)).astype(np.float32),
        "fc2_b": np.zeros(D, np.float32),
    }
    out = kernel(**fake)
    print("kernel ran, out shape", out.shape)
